# revision 5
# baseline (speedup 1.0000x reference)
"""DynamicGate MoE routing kernel for Trainium2 (8 NeuronCores, Bass/Tile).

Computes, for x[N,H], sim_matrix[H,E], gates[E]:
    logits = l2norm_rows(x) @ l2norm_cols(sim_matrix)
    thr    = sigmoid(gates)
    gated  = relu(logits - thr)
    mask   = (gated > 0), with top-1 fallback for all-inactive tokens
    probs  = softmax over active experts of gated
Returns (mask, probs, logits), all [N, E] fp32.

Sharding: data-parallel on the token dim across 8 cores (2048 tokens per
core); sim_matrix/gates replicated. No collectives needed.

Strategy (v3):
  - x is shipped TRANSPOSED ([H, tok]) and split into a 3-byte pair on
    the host: hi = fp16(x) and lo = e4m3((x - hi) * 2^11). This cuts the
    dominant HBM read 25% vs fp32 while giving ~16 effective mantissa
    bits, and eliminates every on-device transpose of x (the h dim lands
    directly on partitions).
  - weights are dual-packed: Whi = [fp16(wn) | fp16((wn - fp16(wn))*2^11)]
    is a [128, 128] stationary operand, so the weight-residual correction
    rides in the same PE pass for free (out rows 64:128). A second fp8
    pass adds lo @ e4m3(wn). logits = hi@W16 + 2^-11*(hi@W16r + lo@W8).
  - per-token sumsq reduces over h (= partitions): squares of hi are
    tree-folded across the 16 h-chunks on DVE/ACT, then a ones-vector
    matmul does the 128-partition reduce into PSUM [1, tok].
  - logits^T and the sumsq row are transposed back to [tok, 65] blocks in
    one pass (65-row transposes), then the epilogue (argmax one-hot,
    threshold mask, masked softmax) runs in natural layout; bf16 DMA-out.
"""

import sys

if "/opt/trn_rl_repo" not in sys.path:
    sys.path.insert(0, "/opt/trn_rl_repo")

import ml_dtypes
import numpy as np

import concourse.bacc as bacc
import concourse.mybir as mybir
from concourse import bass_utils, masks
from concourse.tile import TileContext

F32 = mybir.dt.float32
F32R = mybir.dt.float32r
F16 = mybir.dt.float16
F8 = mybir.dt.float8e4
BF16 = mybir.dt.bfloat16
I32 = mybir.dt.int32
OP = mybir.AluOpType
AF = mybir.ActivationFunctionType
AX = mybir.AxisListType

N, H, E = 16384, 2048, 64
NCORES = 8
NLOC = N // NCORES     # 2048 tokens per core
HC = H // 128          # 16 h-chunks
TB = 512               # tokens per tile
NBLK = TB // 128       # 4 blocks of 128 tokens per tile
NTILE = NLOC // TB     # 4 tiles per core
EPS = 1e-12
RSC = 1.0 / 2048.0     # residual scale 2^-11
MAGIC = 0x5F3759DF


def build():
    nc = bacc.Bacc("TRN2", target_bir_lowering=False, debug=False)
    xhi_d = nc.dram_tensor("xhi", [H, NLOC], F16, kind="ExternalInput")
    xlo_d = nc.dram_tensor("xlo", [H, NLOC], F8, kind="ExternalInput")
    sim_d = nc.dram_tensor("sim", [H, E], F32, kind="ExternalInput")
    gates_d = nc.dram_tensor("gates", [1, E], F32, kind="ExternalInput")
    mask_d = nc.dram_tensor("mask", [NLOC, E], BF16, kind="ExternalOutput")
    probs_d = nc.dram_tensor("probs", [NLOC, E], BF16, kind="ExternalOutput")
    logits_d = nc.dram_tensor("logits", [NLOC, E], BF16, kind="ExternalOutput")

    with TileContext(nc) as tc:
        with (
            tc.tile_pool(name="const", bufs=1) as constp,
            tc.tile_pool(name="xin", bufs=3) as xinp,
            tc.tile_pool(name="sq", bufs=2) as sqp,
            tc.tile_pool(name="lg", bufs=2) as lgp,
            tc.tile_pool(name="ep", bufs=2) as epp,
            tc.tile_pool(name="sc", bufs=2) as scp,
            tc.tile_pool(name="psA", bufs=2, space="PSUM") as psA,
            tc.tile_pool(name="psL", bufs=2, space="PSUM") as psL,
            tc.tile_pool(name="psQ", bufs=2, space="PSUM") as psQ,
            tc.tile_pool(name="psB", bufs=2, space="PSUM") as psB,
        ):
            # ---- x prefetch (first, so DMA engines start immediately) ----
            x_tiles = {}
            next_pf = [0]

            def prefetch():
                ti = next_pf[0]
                if ti >= NTILE:
                    return
                next_pf[0] += 1
                t0 = ti * TB
                hi_t = xinp.tile([128, HC, TB], F16, name="xhi_t", tag="xhi_t")
                nc.sync.dma_start(
                    out=hi_t,
                    in_=xhi_d.ap()[:, t0:t0 + TB].rearrange(
                        "(c p) t -> p c t", p=128),
                )
                lo_t = xinp.tile([128, HC, TB], F8, name="xlo_t", tag="xlo_t")
                nc.sync.dma_start(
                    out=lo_t,
                    in_=xlo_d.ap()[:, t0:t0 + TB].rearrange(
                        "(c p) t -> p c t", p=128),
                )
                x_tiles[ti] = (hi_t, lo_t)

            prefetch()

            # ---- constants ----------------------------------------------
            ident_f = constp.tile([128, 128], F32, name="ident_f")
            masks.make_identity(nc, ident_f)
            onesc = constp.tile([128, 1], F32, name="onesc")
            nc.gpsimd.memset(onesc, 1.0)
            onesr = constp.tile([1, 128], F32, name="onesr")
            nc.gpsimd.memset(onesr, 1.0)
            ones_r = constp.tile([128, 1], F32R, name="ones_r")
            nc.vector.tensor_copy(ones_r, onesc)

            wn = constp.tile([128, HC * E], F32, name="wn")
            g_row = constp.tile([1, E], F32, name="g_row")
            nc.sync.dma_start(
                out=wn.rearrange("p (c e) -> p c e", e=E),
                in_=sim_d.ap().rearrange("(c p) e -> p c e", p=128),
            )
            nc.sync.dma_start(out=g_row, in_=gates_d.ap())

            prefetch()

            def emit_rsqrt(pool, src_ap, shape, tag, f_used=None):
                """rx = 1/sqrt(src) on DVE only: magic-constant + 2 Newton."""
                p, f = shape
                fu = f if f_used is None else f_used
                sa = src_ap[:, 0:fu]
                it = pool.tile([p, f], I32, name=tag + "_i",
                               tag=tag + "_i")[:, 0:fu]
                nc.vector.tensor_scalar(
                    out=it, in0=sa.bitcast(I32), scalar1=1, scalar2=None,
                    op0=OP.logical_shift_right,
                )
                nc.vector.tensor_scalar(
                    out=it, in0=it, scalar1=0xFFFFFFFF, scalar2=None,
                    op0=OP.bitwise_xor,
                )
                nc.vector.tensor_scalar(
                    out=it, in0=it, scalar1=MAGIC + 1, scalar2=None,
                    op0=OP.add,
                )
                y = it.bitcast(F32)
                t1 = pool.tile([p, f], F32, name=tag + "_t",
                               tag=tag + "_t")[:, 0:fu]
                for _ in range(2):
                    nc.vector.tensor_tensor(out=t1, in0=y, in1=y, op=OP.mult)
                    nc.vector.tensor_tensor(out=t1, in0=t1, in1=sa, op=OP.mult)
                    nc.vector.tensor_scalar(
                        out=t1, in0=t1, scalar1=-0.5, scalar2=1.5,
                        op0=OP.mult, op1=OP.add,
                    )
                    nc.vector.tensor_tensor(out=y, in0=y, in1=t1, op=OP.mult)
                return y

            # Whi = [fp16(wn) | fp16((wn - fp16(wn)) * 2^11)], W8 = e4m3(wn)
            Whi = constp.tile([128, HC, 128], F16, name="Whi")
            W8 = constp.tile([128, HC, E], F8, name="W8")
            thr_bb = constp.tile([128, E], BF16, name="thr_bb")

            def emit_wn_preamble():
                wnsq = constp.tile([128, HC * E], F32, name="wnsq")
                nc.scalar.square(wnsq, wn)
                csb = psB.tile([128, NBLK, 65], F32, name="csb", tag="ptb")
                cs_ps = csb.rearrange("p j e -> p (j e)")[0:1, 0:E]
                for c in range(HC):
                    nc.tensor.matmul(
                        cs_ps, lhsT=onesc, rhs=wnsq[:, c * E:(c + 1) * E],
                        start=(c == 0), stop=(c == HC - 1),
                    )
                # rwn = 1/max(sqrt(cs), EPS): DVE-only Newton rsqrt
                csm = constp.tile([1, E], F32, name="csm")
                nc.vector.tensor_scalar(
                    out=csm, in0=cs_ps, scalar1=EPS * EPS, scalar2=None,
                    op0=OP.max,
                )
                rwn = emit_rsqrt(constp, csm, (1, E), "rwn")

                # thr = sigmoid(g) = 1/(1+exp(-g))
                eneg = constp.tile([1, E], F32, name="eneg")
                nc.scalar.activation(eneg, g_row, AF.Exp, scale=-1.0)
                nc.vector.tensor_scalar(
                    out=eneg, in0=eneg, scalar1=1.0, scalar2=None, op0=OP.add
                )
                thr_row = constp.tile([1, E], F32, name="thr_row")
                nc.vector.reciprocal(thr_row, eneg)

                # broadcast [1,E] rows to 128 partitions via rank-1 matmul
                bcb = psB.tile([128, NBLK, 65], F32, name="bcb", tag="ptb")
                bc_ps = bcb.rearrange("p j e -> p (j e)")[:, 0:2 * E]
                nc.tensor.matmul(bc_ps[:, 0:E], lhsT=onesr, rhs=rwn,
                                 start=True, stop=True)
                nc.tensor.matmul(bc_ps[:, E:2 * E], lhsT=onesr, rhs=thr_row,
                                 start=True, stop=True)
                rwn_b = constp.tile([128, E], F32, name="rwn_b")
                nc.scalar.copy(rwn_b, bc_ps[:, 0:E])
                nc.scalar.copy(thr_bb, bc_ps[:, E:2 * E])

                # wnf[p, c, e] = wn * rwn (column-normalized, full fp32)
                wnf = constp.tile([128, HC, E], F32, name="wnf")
                nc.vector.tensor_tensor(
                    out=wnf,
                    in0=wn.rearrange("p (c e) -> p c e", e=E),
                    in1=rwn_b.unsqueeze(1).broadcast_to([128, HC, E]),
                    op=OP.mult,
                )
                # W16 = fp16(wnf); W16r = fp16((wnf - W16) * 2^11); W8 = e4m3
                nc.vector.tensor_copy(Whi[:, :, 0:E], wnf)
                nc.vector.tensor_copy(W8, wnf)
                wup = constp.tile([128, HC, E], F32, name="wup")
                nc.vector.tensor_copy(wup, Whi[:, :, 0:E])
                nc.vector.tensor_tensor(
                    out=wup, in0=wnf, in1=wup, op=OP.subtract)
                nc.vector.tensor_scalar(
                    out=Whi[:, :, E:2 * E], in0=wup, scalar1=2048.0,
                    scalar2=None, op0=OP.mult,
                )

            emit_wn_preamble()

            def emit_epilogue(t0, ptb, rx):
                # -- epilogue on [128, NBLK, E] natural-layout blocks ------
                def bce(ap):   # [128, NBLK] -> [128, NBLK, E] stride-0
                    return ap.unsqueeze(2).broadcast_to([128, NBLK, E])

                pts = ptb[:, :, 0:E]
                lmax = scp.tile([128, NBLK], F32, name="lmax",
                                tag="lmax")
                nc.vector.tensor_reduce(
                    out=lmax, in_=pts, axis=AX.X, op=OP.max,
                )
                onehot = epp.tile([128, NBLK, E], BF16, name="onehot",
                                  tag="onehot")
                nc.vector.tensor_tensor(
                    out=onehot, in0=pts, in1=bce(lmax), op=OP.is_equal,
                )
                logits_bf = epp.tile([128, NBLK, E], BF16, name="logits_bf",
                                     tag="logits_bf")
                nc.vector.tensor_tensor(
                    out=logits_bf, in0=pts, in1=bce(rx), op=OP.mult,
                )
                gsub = epp.tile([128, NBLK, E], BF16, name="gsub",
                                tag="gsub")
                nc.vector.tensor_tensor(
                    out=gsub, in0=logits_bf,
                    in1=thr_bb.unsqueeze(1).broadcast_to([128, NBLK, E]),
                    op=OP.subtract,
                )
                ind = epp.tile([128, NBLK, E], BF16, name="ind",
                               tag="ind")
                nc.vector.tensor_scalar(
                    out=ind, in0=gsub, scalar1=0.0, scalar2=None,
                    op0=OP.is_gt,
                )
                nact = scp.tile([128, NBLK], F32, name="nact", tag="nact")
                nc.vector.tensor_reduce(
                    out=nact, in_=ind, axis=AX.X, op=OP.add,
                )
                inact = scp.tile([128, NBLK], F32, name="inact",
                                 tag="inact")
                nc.vector.tensor_scalar(
                    out=inact, in0=nact, scalar1=0.0, scalar2=None,
                    op0=OP.is_equal,
                )
                maskt = epp.tile([128, NBLK, E], BF16, name="maskt",
                                 tag="maskt")
                nc.gpsimd.tensor_tensor(
                    out=maskt, in0=onehot, in1=bce(inact), op=OP.mult,
                )
                nc.gpsimd.tensor_tensor(
                    out=maskt, in0=maskt, in1=ind, op=OP.add,
                )
                # probs = mask*exp(gsub) / sum(mask*exp(gsub))
                ex = epp.tile([128, NBLK, E], BF16, name="ex", tag="ex")
                nc.scalar.activation(ex, gsub, AF.Exp)
                me = epp.tile([128, NBLK, E], BF16, name="me", tag="me")
                nc.vector.tensor_tensor(
                    out=me, in0=ex, in1=maskt, op=OP.mult,
                )
                sesum = scp.tile([128, NBLK], F32, name="sesum",
                                 tag="sesum")
                nc.vector.tensor_reduce(
                    out=sesum, in_=me, axis=AX.X, op=OP.add,
                )
                rs = scp.tile([128, NBLK], F32, name="rs", tag="rs")
                nc.vector.reciprocal(rs, sesum)
                probs = epp.tile([128, NBLK, E], BF16, name="probs",
                                 tag="probs")
                nc.vector.tensor_tensor(
                    out=probs, in0=me, in1=bce(rs), op=OP.mult,
                )

                gtok = slice(t0, t0 + TB)
                for out_d, osrc in ((mask_d, maskt), (probs_d, probs),
                                    (logits_d, logits_bf)):
                    nc.sync.dma_start(
                        out=out_d.ap()[gtok, :].rearrange(
                            "(j p) e -> p j e", p=128),
                        in_=osrc,
                    )

            # ---- main loop: 4 tiles of 512 tokens ------------------------
            pending = None
            for ti in range(NTILE):
                hi_t, lo_t = x_tiles.pop(ti)
                prefetch()

                # -- sumsq: squares + tree-fold over h-chunks (DVE/ACT) ----
                t1 = sqp.tile([128, 8, TB], F16, name="sq1", tag="sq1")
                nc.vector.tensor_tensor(
                    out=t1, in0=hi_t[:, 0:8, :], in1=hi_t[:, 0:8, :],
                    op=OP.mult,
                )
                t2 = sqp.tile([128, 8, TB], F16, name="sq2", tag="sq2")
                nc.scalar.square(t2, hi_t[:, 8:16, :])
                s8 = sqp.tile([128, 8, TB], F16, name="s8", tag="s8")
                nc.vector.tensor_tensor(out=s8, in0=t1, in1=t2, op=OP.add)
                s4 = sqp.tile([128, 4, TB], F16, name="s4", tag="s4")
                nc.vector.tensor_tensor(
                    out=s4, in0=s8[:, 0:4, :], in1=s8[:, 4:8, :], op=OP.add)
                s2 = sqp.tile([128, 2, TB], F16, name="s2", tag="s2")
                nc.vector.tensor_tensor(
                    out=s2, in0=s4[:, 0:2, :], in1=s4[:, 2:4, :], op=OP.add)
                psqacc = sqp.tile([128, TB], F32R, name="psqacc",
                                  tag="psqacc")
                nc.vector.tensor_tensor(
                    out=psqacc, in0=s2[:, 0, :], in1=s2[:, 1, :], op=OP.add)

                # -- PE: hi pass (dual-packed W), lo pass, sumsq reduce ----
                pshi = psA.tile([128, TB], F32, name="pshi", tag="pshi")
                for c in range(HC):
                    nc.tensor.matmul(
                        pshi, lhsT=Whi[:, c, :], rhs=hi_t[:, c, :],
                        start=(c == 0), stop=(c == HC - 1),
                    )
                pslo = psL.tile([64, TB], F32, name="pslo", tag="pslo")
                for c in range(HC):
                    nc.tensor.matmul(
                        pslo, lhsT=W8[:, c, :], rhs=lo_t[:, c, :],
                        start=(c == 0), stop=(c == HC - 1),
                    )
                psq1 = psQ.tile([1, TB], F32, name="psq1", tag="psq1")
                nc.tensor.matmul(psq1, lhsT=ones_r, rhs=psqacc,
                                 start=True, stop=True)

                # -- combine into lgs [65, TB]: rows 0:64 logits, 64 ssq ---
                # (each op reads at most one PSUM operand: NCC_IBVF027)
                lgs = lgp.tile([65, TB], F32, name="lgs", tag="lgs")
                nc.vector.tensor_scalar(
                    out=lgs[0:64, :], in0=pslo, scalar1=RSC, scalar2=None,
                    op0=OP.mult,
                )
                nc.vector.scalar_tensor_tensor(
                    out=lgs[0:64, :], in0=pshi[64:128, :], scalar=RSC,
                    in1=lgs[0:64, :], op0=OP.mult, op1=OP.add,
                )
                nc.vector.tensor_tensor(
                    out=lgs[0:64, :], in0=pshi[0:64, :], in1=lgs[0:64, :],
                    op=OP.add,
                )
                nc.scalar.copy(lgs[64:65, :], psq1)

                # -- transpose [65, 128] blocks back to natural layout -----
                ptb = psB.tile([128, NBLK, 65], F32, name="ptb", tag="ptb")
                for j in range(NBLK):
                    nc.tensor.transpose(
                        ptb[:, j, :], lgs[:, j * 128:(j + 1) * 128],
                        ident_f[0:65, 0:65],
                    )
                # rx = 1/max(sqrt(ssq), eps) from the transposed ssq column
                ssqm = scp.tile([128, NBLK], F32, name="ssqm", tag="ssqm")
                nc.vector.tensor_scalar(
                    out=ssqm, in0=ptb[:, :, 64], scalar1=EPS * EPS,
                    scalar2=None, op0=OP.max,
                )
                rx = emit_rsqrt(scp, ssqm, (128, NBLK), "rx")

                # previous tile's epilogue drains while this tile streams
                if pending is not None:
                    emit_epilogue(*pending)
                pending = (ti * TB, ptb, rx)
            emit_epilogue(*pending)

    nc.compile()
    return nc


_NC_CACHE = {}


def _get_nc():
    if "nc" not in _NC_CACHE:
        _NC_CACHE["nc"] = build()
    return _NC_CACHE["nc"]


def make_in_maps(x, sim_matrix, gates):
    x = np.asarray(x, dtype=np.float32)
    hi = x.astype(np.float16)
    lo = ((x - hi.astype(np.float32)) * 2048.0).astype(ml_dtypes.float8_e4m3)
    sim = np.ascontiguousarray(np.asarray(sim_matrix, dtype=np.float32))
    g = np.ascontiguousarray(np.asarray(gates, dtype=np.float32)).reshape(1, E)
    maps = []
    for c in range(NCORES):
        sl = slice(c * NLOC, (c + 1) * NLOC)
        maps.append({
            "xhi": np.ascontiguousarray(hi[sl].T),
            "xlo": np.ascontiguousarray(lo[sl].T),
            "sim": sim,
            "gates": g,
        })
    return maps


def kernel(x, sim_matrix, gates):
    nc = _get_nc()
    in_maps = make_in_maps(x, sim_matrix, gates)
    res = bass_utils.run_bass_kernel_spmd(nc, in_maps, core_ids=list(range(NCORES)))
    outs = []
    for name in ("mask", "probs", "logits"):
        outs.append(np.concatenate(
            [np.asarray(res.results[c][name], dtype=np.float32)
             for c in range(NCORES)], axis=0))
    return tuple(outs)
